# revision 1
# baseline (speedup 1.0000x reference)
"""Trainium2 Bass kernel for nn_MHA_42391327211690.

MHA: B=1, S=2048, E=2048, H=32 q-heads, HKV=8 kv-heads, D=64, RoPE(rot=64,
GPT-NeoX style) on q/k, causal GQA attention, out-projection with bias.

Distribution (8 NeuronCores, tensor-parallel by heads):
  - core i computes q-heads 4i..4i+3 and kv-head i (Wqkv column-sharded),
  - attention entirely local (GQA groups align with the shard),
  - AllToAll redistributes ctx^T from head-sharded to sequence-sharded,
  - out-projection computed per-core for its 256-row sequence slice
    (weights replicated), host concatenates the slices.

All matmuls run as float32r (FP22 mantissa-truncated fp32, full PE rate).
Scores layout is transposed ([t, sq]) so softmax normalization needs no
transposes: denominator comes from an appended ones-column in V, applied
as a reciprocal multiply on the ctx^T tile.
"""

from contextlib import ExitStack

import numpy as np

import concourse.bass as bass
import concourse.bacc as bacc
import concourse.tile as tile
from concourse import mybir
from concourse.bass_utils import run_bass_kernel_spmd

F32 = mybir.dt.float32
F32R = mybir.dt.float32r
AF = mybir.ActivationFunctionType
ALU = mybir.AluOpType

B, E = 1, 2048
H, HKV, D = 32, 8, 64
ROT, BASE = 64, 10000.0
NCORES = 8
HL = H // NCORES            # 4 local q heads
OPL = (HL + 2) * D          # 384 local qkv output rows (q | k | v)
SCALE = float(D) ** -0.5


def _r(x):
    return x.bitcast(F32R)


def build_nc(S=2048, n_cores=NCORES):
    """Build the SPMD Bass program (identical on every core)."""
    SEG = S // n_cores      # per-core output sequence slice
    NT = S // 128           # t-blocks (key blocks)
    NG = S // 512           # sq groups of 512
    NE = E // 128           # contraction tiles for qkv / out proj
    FD = HL * D             # 256 local ctx rows

    nc = bacc.Bacc("TRN2", target_bir_lowering=False, debug=False,
                   num_devices=n_cores)

    xT = nc.dram_tensor("xT", [E, S], F32, kind="ExternalInput")
    wqkvT = nc.dram_tensor("wqkvT", [E, OPL], F32, kind="ExternalInput")
    bqkv = nc.dram_tensor("bqkv", [OPL, 1], F32, kind="ExternalInput")
    cc_d = nc.dram_tensor("cc", [128, S], F32, kind="ExternalInput")
    ss_d = nc.dram_tensor("ss", [128, S], F32, kind="ExternalInput")
    triu_d = nc.dram_tensor("triu", [128, 128], F32, kind="ExternalInput")
    id_d = nc.dram_tensor("ident", [128, 64], F32, kind="ExternalInput")
    woT = nc.dram_tensor("woT", [E, E], F32, kind="ExternalInput")
    outb_d = nc.dram_tensor("outb", [128, E], F32, kind="ExternalInput")
    outS = nc.dram_tensor("outS", [SEG, E], F32, kind="ExternalOutput")

    with tile.TileContext(nc) as tc, ExitStack() as ctx:
        wo_sb = {}

        ab = ExitStack()
        ctx_pool = ab.enter_context(tc.tile_pool(name="ctxsb", bufs=1))
        ctx_sb = [ctx_pool.tile([64, S], F32, tag=f"c{i}", name=f"ctxsb{i}") for i in range(HL)]
        consts = ab.enter_context(tc.tile_pool(name="consts", bufs=1))
        triu = consts.tile([128, 128], F32)
        bq = [consts.tile([128, 1], F32, tag=f"bq{j}", name=f"bq{j}") for j in range(3)]

        # persistent qkv activations (phases A+B)
        qkv_pool = ab.enter_context(tc.tile_pool(name="qkv", bufs=1))
        q_sb = [qkv_pool.tile([128, S], F32, tag=f"q{i}", name=f"qsb{i}") for i in range(HL // 2)]
        kv_sb = qkv_pool.tile([128, S], F32, tag="kv")      # k rows 0:64, v rows 64:128
        kdup = qkv_pool.tile([128, S], F32, tag="kdup")     # roped k duplicated
        v_pool = ab.enter_context(tc.tile_pool(name="vsb", bufs=1))
        v_sb = [v_pool.tile([128, D + 1], F32R, tag=f"v{t}", name=f"vsb{t}") for t in range(NT)]

        # ---------------- Phase A: QKV + RoPE + V transpose -------------
        with tc.tile_pool(name="xw", bufs=1) as xw_pool, \
             tc.tile_pool(name="ropet", bufs=2) as rope_pool, \
             tc.tile_pool(name="psqkv", bufs=2, space="PSUM") as ps_qkv, \
             tc.tile_pool(name="psvt", bufs=2, space="PSUM") as ps_vt:

            cc = xw_pool.tile([128, S], F32)
            ss = xw_pool.tile([128, S], F32)
            ident = xw_pool.tile([128, 64], F32)
            wq_sb = [xw_pool.tile([128, OPL], F32R, tag=f"wq{e}", name=f"wqsb{e}") for e in range(NE)]
            # x^T streamed per s-group as [128, 512] column slices (each
            # byte read once); double-buffered so s-group sg+1's slices load
            # during sg's matmuls. One PSUM accumulation over all 16 e-tiles.
            xs = {}

            def _x_slices(sg):
                for e in range(NE):
                    xs[(sg, e)] = xw_pool.tile([128, 512], F32R, tag=f"x{e}",
                                               bufs=2, name=f"xs{sg}_{e}")
                    nc.sync.dma_start(
                        xs[(sg, e)][:],
                        xT[e * 128:(e + 1) * 128,
                           sg * 512:(sg + 1) * 512].bitcast(F32R))

            _x_slices(0)
            for e in range(NE):
                nc.sync.dma_start(wq_sb[e][:], wqkvT[e * 128:(e + 1) * 128, :].bitcast(F32R))
            for j in range(3):
                nc.sync.dma_start(bq[j][:], bqkv[j * 128:(j + 1) * 128, :])
            nc.sync.dma_start(cc[:], cc_d[:])
            nc.sync.dma_start(ss[:], ss_d[:])
            nc.sync.dma_start(triu[:], triu_d[:])
            nc.sync.dma_start(ident[:], id_d[:])

            # warm the ACT exp table so the first attention exp call does
            # not pay the ~2.7us ACT_TABLE_LOAD at the QKV->attention boundary
            warm = xw_pool.tile([128, 1], F32, tag="warm")
            nc.scalar.activation(warm[:], bq[0][:], AF.Exp, scale=0.0)

            NSG = S // 512
            for sg in range(NSG):
                if sg + 1 < NSG:
                    _x_slices(sg + 1)
                sgs = slice(sg * 512, (sg + 1) * 512)
                ps = ps_qkv.tile([128, 1536], F32, tag="qkvps")
                for e in range(NE):
                    for j in range(3):
                        nc.tensor.matmul(
                            ps[:, j * 512:(j + 1) * 512],
                            _r(wq_sb[e][:, j * 128:(j + 1) * 128]),
                            _r(xs[(sg, e)][:]),
                            start=(e == 0), stop=(e == NE - 1))
                dsts = [q_sb[0], q_sb[1], kv_sb]
                for j in range(3):
                    # q tiles are consumed by fp32r matmuls; every write
                    # to them must carry the fp32r rounding tag
                    out_ap = dsts[j][:, sgs]
                    if j < 2:
                        out_ap = out_ap.bitcast(F32R)
                    nc.scalar.activation(
                        out_ap, ps[:, j * 512:(j + 1) * 512],
                        AF.Identity, bias=bq[j][:], scale=1.0)
                if True:
                    # ---- RoPE: swapped halves built via SBUF->SBUF DMA
                    # (DVE requires equal base partitions on both inputs)
                    for qi in range(HL // 2):
                        qt = q_sb[qi]
                        qs = rope_pool.tile([128, 512], F32, tag="qs")
                        for b in range(4):
                            nc.gpsimd.dma_start(
                                qs[b * 32:(b + 1) * 32, :],
                                qt[(b ^ 1) * 32:((b ^ 1) + 1) * 32, sgs])
                        t1 = rope_pool.tile([128, 512], F32, tag="t1")
                        t2 = rope_pool.tile([128, 512], F32, tag="t2")
                        nc.vector.tensor_mul(t1[:], qt[:, sgs], cc[:, sgs])
                        nc.vector.tensor_mul(t2[:], qs[:], ss[:, sgs])
                        nc.vector.tensor_add(qt[:, sgs].bitcast(F32R), t1[:], t2[:])
                    # k: rows 0:64 of kv_sb -> roped into kdup, then dup'd
                    ks = rope_pool.tile([64, 512], F32, tag="ks")
                    for b in range(2):
                        nc.gpsimd.dma_start(
                            ks[b * 32:(b + 1) * 32, :],
                            kv_sb[(b ^ 1) * 32:((b ^ 1) + 1) * 32, sgs])
                    t1 = rope_pool.tile([64, 512], F32, tag="kt1")
                    t2 = rope_pool.tile([64, 512], F32, tag="kt2")
                    nc.vector.tensor_mul(t1[:], kv_sb[0:64, sgs], cc[0:64, sgs])
                    nc.vector.tensor_mul(t2[:], ks[:], ss[0:64, sgs])
                    nc.vector.tensor_add(kdup[0:64, sgs].bitcast(F32R), t1[:], t2[:])
                    nc.gpsimd.dma_start(kdup[64:128, sgs].bitcast(F32R),
                                      kdup[0:64, sgs].bitcast(F32R))
                    # ---- V transpose: kv_sb rows 64:128 -> v_sb tiles ----
                    for c in range(4):
                        t = sg * 4 + c
                        pvt = ps_vt.tile([128, 64], F32, tag="vt")
                        nc.tensor.transpose(
                            pvt[:],
                            kv_sb[64:128, sg * 512 + c * 128: sg * 512 + (c + 1) * 128],
                            ident[64:128, :])
                        nc.vector.memset(v_sb[t][:, 64:65].bitcast(F32), 1.0)
                        nc.vector.tensor_copy(v_sb[t][:, 0:64], pvt[:])

        # output-projection weights pool (created after the x/wqkv pool is
        # released so its SBUF region does not overlap phase A's). Weights
        # stream in 512-column quarters, double-buffered per f-tile; the
        # first two quarters are DMA'd during the second attention head-pair.
        wo_pool = ctx.enter_context(tc.tile_pool(name="wo", bufs=2, side="right"))

        def _wo_quarter(eg, pool):
            for f in range(NE):
                w = pool.tile([128, 512], F32R, tag=f"wo{f}_{eg}",
                              name=f"wo{f}_{eg}", bufs=1)
                wo_sb[(f, eg)] = w
                nc.sync.dma_start(
                    w[:], woT[f * 128:(f + 1) * 128,
                              eg * 512:(eg + 1) * 512].bitcast(F32R))

        # ---------------- Phase B: attention --------------------------
        with tc.tile_pool(name="exps", bufs=2) as exps_pool, \
             tc.tile_pool(name="rcp", bufs=2) as rcp_pool, \
             tc.tile_pool(name="pss", bufs=2, space="PSUM") as ps_s, \
             tc.tile_pool(name="psctx", bufs=2, space="PSUM") as ps_ctx:
            dram = ctx.enter_context(tc.tile_pool(name="dram", bufs=1, space="DRAM"))
            a2a_in = [dram.tile([n_cores * 128, SEG], F32, tag=f"ai{p}", name=f"a2ain{p}")
                      for p in range(2)]
            a2a_out = [dram.tile([n_cores * 128, SEG], F32, tag=f"ao{p}", name=f"a2aout{p}")
                       for p in range(2)]
            for hp in range(HL // 2):
                if hp == 1:
                    _wo_quarter(0, wo_pool)
                qt = q_sb[hp]
                for g in range(NG):
                    gs = slice(g * 512, (g + 1) * 512)
                    pc_e = ps_ctx.tile([D + 1, 512], F32, tag="ctx_e")
                    pc_o = ps_ctx.tile([D + 1, 512], F32, tag="ctx_o")
                    ntb = 4 * g + 4
                    for t in range(ntb):
                        ts_ = slice(t * 128, (t + 1) * 128)
                        pss = ps_s.tile([128, 1024], F32, tag="s")
                        nc.tensor.matmul(pss[:, 0:512], _r(kdup[0:64, ts_]),
                                         _r(qt[0:64, gs]), start=True, stop=True)
                        nc.tensor.matmul(pss[:, 512:1024], _r(kdup[64:128, ts_]),
                                         _r(qt[64:128, gs]), start=True, stop=True)
                        ex = exps_pool.tile([128, 1024], F32R, tag="e")
                        nc.scalar.activation(ex[:], pss[:], AF.Exp, scale=SCALE)
                        j = t - 4 * g
                        if j > 0:
                            nc.vector.memset(ex[:, 0:j * 128].bitcast(F32), 0.0)
                            nc.vector.memset(ex[:, 512:512 + j * 128].bitcast(F32), 0.0)
                        if j >= 0:
                            for h2 in range(2):
                                sl = slice(h2 * 512 + j * 128, h2 * 512 + (j + 1) * 128)
                                nc.vector.tensor_mul(ex[:, sl], ex[:, sl], triu[:])
                        nc.tensor.matmul(pc_e[:], _r(v_sb[t][:]), _r(ex[:, 0:512]),
                                         start=(t == 0), stop=(t == ntb - 1))
                        nc.tensor.matmul(pc_o[:], _r(v_sb[t][:]), _r(ex[:, 512:1024]),
                                         start=(t == 0), stop=(t == ntb - 1))
                    # copy ctx+den to SBUF immediately so the PSUM banks free
                    cu = rcp_pool.tile([D + 1, 1024], F32, tag="cu")
                    nc.vector.tensor_copy(cu[:, 0:512], pc_e[:])
                    nc.vector.tensor_copy(cu[:, 512:1024], pc_o[:])
                    den = rcp_pool.tile([1, 1024], F32, tag="den", bufs=1)
                    nc.vector.tensor_copy(den[:], cu[64:65, :])
                    # 1/den (2-pass NR reciprocal), broadcast across the 64
                    # ctx partitions by doubling SBUF->SBUF DMAs — no PE use,
                    # so attention matmuls are never head-of-line blocked
                    rb = rcp_pool.tile([64, 1024], F32, tag="rb")
                    rscr = rcp_pool.tile([1, 1024], F32, tag="rscr", bufs=1)
                    nc.vector.reciprocal_approx_accurate(rb[0:1, :], den[:],
                                                         rscr[:])
                    k = 1
                    while k < 64:
                        nc.gpsimd.dma_start(rb[k:2 * k, :], rb[0:k, :])
                        k *= 2
                    nc.vector.tensor_mul(ctx_sb[2 * hp][:, gs],
                                         cu[0:64, 0:512], rb[:, 0:512])
                    nc.vector.tensor_mul(ctx_sb[2 * hp + 1][:, gs],
                                         cu[0:64, 512:1024], rb[:, 512:1024])
                # this head-pair's ctx rows are final: ship its AllToAll now
                # so the collective overlaps the next head-pair's attention
                for j in range(n_cores):
                    for h2 in range(2):
                        nc.gpsimd.dma_start(
                            a2a_in[hp][j * 128 + h2 * 64: j * 128 + (h2 + 1) * 64, :],
                            ctx_sb[2 * hp + h2][:, j * SEG:(j + 1) * SEG])
                with tc.high_priority():
                    nc.gpsimd.collective_compute(
                        "AllToAll", ALU.bypass,
                        replica_groups=[list(range(n_cores))],
                        ins=[a2a_in[hp][:]], outs=[a2a_out[hp][:]])

        ab.close()  # release qkv/v/consts SBUF before the out-projection

        # ---------------- Phase C: out projection ---------------------
        with tc.tile_pool(name="cf", bufs=1) as cf_pool, \
             tc.tile_pool(name="osb", bufs=1) as out_pool, \
             tc.tile_pool(name="woc", bufs=1, side="right") as woc_pool, \
             tc.tile_pool(name="pso", bufs=2, space="PSUM") as ps_o:
            outb = cf_pool.tile([128, E], F32)
            nc.sync.dma_start(outb[:], outb_d[:])
            ctxF = [cf_pool.tile([128, SEG], F32R, tag=f"cf{f}", name=f"cfsb{f}") for f in range(NE)]

            def _ctxF(par):
                for f in range(par, NE, 2):
                    c = f // 2
                    nc.sync.dma_start(ctxF[f][:],
                                      a2a_out[par][c * 128:(c + 1) * 128, :].bitcast(F32R))

            _wo_quarter(1, woc_pool)
            _ctxF(0)
            _wo_quarter(2, woc_pool)
            _wo_quarter(3, woc_pool)
            _ctxF(1)
            out_sb = [out_pool.tile([128, E], F32, tag=f"ot{s}", name=f"osb{s}")
                      for s in range(SEG // 128)]
            # pass 1: head-pair-0 partial products (only need A2A #0), added
            # into out_sb with the bias; pass 2 accumulates head-pair-1 on
            # top once A2A #1 has landed. This keeps the PE busy during the
            # second collective.
            groups = [(ehalf, egl, st) for ehalf in range(2)
                      for egl in range(2) for st in range(SEG // 128)]
            for p in range(2):
                for (ehalf, egl, st) in groups:
                    eg = ehalf * 2 + egl
                    po = ps_o.tile([128, 512], F32, tag=f"o{p}")
                    fs = [f for f in range(NE) if f % 2 == p]
                    for fi, f in enumerate(fs):
                        nc.tensor.matmul(
                            po[:],
                            _r(ctxF[f][:, st * 128:(st + 1) * 128]),
                            wo_sb[(f, eg)][:],
                            start=(fi == 0), stop=(fi == len(fs) - 1))
                    osl = out_sb[st][:, eg * 512:(eg + 1) * 512]
                    if p == 0:
                        nc.vector.scalar_tensor_tensor(
                            osl, po[:], 1.0,
                            outb[:, eg * 512:(eg + 1) * 512], ALU.mult, ALU.add)
                    else:
                        nc.vector.scalar_tensor_tensor(
                            osl, po[:], 1.0, osl, ALU.mult, ALU.add)
            for st in range(SEG // 128):
                nc.sync.dma_start(outS[st * 128:(st + 1) * 128, :], out_sb[st][:])

    nc.compile()
    return nc


def shard_inputs(hidden_states, Wqkv_w, Wqkv_b, out_w, out_b, S=2048,
                 n_cores=NCORES):
    """Host-side sharding: returns per-core input maps."""
    x = np.asarray(hidden_states, np.float32).reshape(S, E)
    xT = np.ascontiguousarray(x.T)
    Wqkv_w = np.asarray(Wqkv_w, np.float32)
    Wqkv_b = np.asarray(Wqkv_b, np.float32)
    woT = np.ascontiguousarray(np.asarray(out_w, np.float32).T)
    outb = np.ascontiguousarray(np.broadcast_to(np.asarray(out_b, np.float32).reshape(1, E), (128, E)))

    inv = (1.0 / (BASE ** (np.arange(0, ROT, 2, dtype=np.float64) / ROT)))
    t = np.arange(S, dtype=np.float64)
    freqs = np.outer(t, inv)                      # [S, 32]
    cT = np.cos(freqs).T.astype(np.float32)       # [32, S]
    sT = np.sin(freqs).T.astype(np.float32)
    cc = np.tile(cT, (4, 1))                      # [128, S]
    ss = np.concatenate([-sT, sT, -sT, sT], axis=0)
    triu = (np.arange(128)[:, None] <= np.arange(128)[None, :]).astype(np.float32)
    ident = np.vstack([np.eye(64, dtype=np.float32)] * 2)

    in_maps = []
    for i in range(n_cores):
        hq = H // n_cores
        wq = Wqkv_w[i * hq * D:(i + 1) * hq * D]          # [256, E]
        wk = Wqkv_w[H * D + i * D: H * D + (i + 1) * D]   # [64, E]
        wv = Wqkv_w[(H + HKV) * D + i * D: (H + HKV) * D + (i + 1) * D]
        w_local = np.concatenate([wq, wk, wv], axis=0)    # [384, E]
        b_local = np.concatenate([
            Wqkv_b[i * hq * D:(i + 1) * hq * D],
            Wqkv_b[H * D + i * D: H * D + (i + 1) * D],
            Wqkv_b[(H + HKV) * D + i * D: (H + HKV) * D + (i + 1) * D]])
        in_maps.append({
            "xT": xT,
            "wqkvT": np.ascontiguousarray(w_local.T),
            "bqkv": np.ascontiguousarray(b_local.reshape(OPL, 1)),
            "cc": cc, "ss": ss, "triu": triu, "ident": ident,
            "woT": woT, "outb": outb,
        })
    return in_maps


def assemble(results, S=2048, n_cores=NCORES):
    out = np.concatenate([r["outS"] for r in results], axis=0)
    return out.reshape(B, S, E).astype(np.float32)


_NC_CACHE = {}


def _get_nc(S=2048):
    if S not in _NC_CACHE:
        _NC_CACHE[S] = build_nc(S=S)
    return _NC_CACHE[S]


def kernel(hidden_states, Wqkv_w, Wqkv_b, out_w, out_b, _trace=False):
    in_maps = shard_inputs(hidden_states, Wqkv_w, Wqkv_b, out_w, out_b)
    nc = _get_nc()
    res = run_bass_kernel_spmd(nc, in_maps, core_ids=list(range(NCORES)),
                               trace=_trace)
    out = assemble(res.results)
    if _trace:
        kernel.last_results = res
    return out



# revision 23
# speedup vs baseline: 1.3151x; 1.3151x over previous
"""Trainium2 Bass kernel for nn_MHA_42391327211690.

MHA: B=1, S=2048, E=2048, H=32 q-heads, HKV=8 kv-heads, D=64, RoPE(rot=64,
GPT-NeoX style) on q/k, causal GQA attention, out-projection with bias.

Distribution (8 NeuronCores, tensor-parallel by heads):
  - core i computes q-heads 4i..4i+3 and kv-head i (Wqkv column-sharded),
  - attention entirely local (GQA groups align with the shard),
  - AllToAll redistributes ctx^T from head-sharded to sequence-sharded,
  - out-projection computed per-core for its 256-row sequence slice
    (weights replicated), host concatenates the slices.

v2: all activations/weights in bf16 (host-converted; full PE rate, half
the DMA/SBUF of fp32), out-proj weights fully prefetched during
attention, softmax denominator replicated via 64 ones-columns in the V
tile (AV matmul M=128: rows 0:64 ctx, 64:128 denominator) so the
reciprocal runs on 64 partitions directly, scores/exp restricted to the
valid causal column range per diagonal block.
"""

from contextlib import ExitStack

import numpy as np
import ml_dtypes

import concourse.bass as bass
import concourse.bacc as bacc
import concourse.tile as tile
from concourse import mybir
from concourse.bass_utils import run_bass_kernel_spmd

F32 = mybir.dt.float32
BF16 = mybir.dt.bfloat16
AF = mybir.ActivationFunctionType
ALU = mybir.AluOpType

B, E = 1, 2048
H, HKV, D = 32, 8, 64
ROT, BASE = 64, 10000.0
NCORES = 8
HL = H // NCORES            # 4 local q heads
OPL = (HL + 2) * D          # 384 local qkv output rows (q | k | v)
SCALE = float(D) ** -0.5


def build_nc(S=2048, n_cores=NCORES):
    """Build the SPMD Bass program (identical on every core)."""
    SEG = S // n_cores      # per-core output sequence slice
    NT = S // 128           # t-blocks (key blocks)
    NG = S // 512           # sq groups of 512
    NE = E // 128           # contraction tiles for qkv / out proj

    nc = bacc.Bacc("TRN2", target_bir_lowering=False, debug=False,
                   num_devices=n_cores)

    xT = nc.dram_tensor("xT", [E, S], BF16, kind="ExternalInput")
    wqkvT = nc.dram_tensor("wqkvT", [E, OPL], BF16, kind="ExternalInput")
    bqkv = nc.dram_tensor("bqkv", [OPL, 1], F32, kind="ExternalInput")
    cc_d = nc.dram_tensor("cc", [128, S], BF16, kind="ExternalInput")
    ss_d = nc.dram_tensor("ss", [128, S], BF16, kind="ExternalInput")
    triu_d = nc.dram_tensor("triu", [128, 128], BF16, kind="ExternalInput")
    id_d = nc.dram_tensor("ident", [128, 64], F32, kind="ExternalInput")
    woT = nc.dram_tensor("woT", [E, E], BF16, kind="ExternalInput")
    outb_d = nc.dram_tensor("outb", [128, E], F32, kind="ExternalInput")
    outS = nc.dram_tensor("outS", [SEG, E], F32, kind="ExternalOutput")

    with tile.TileContext(nc) as tc, ExitStack() as ctx:
        wo_sb = {}

        ab = ExitStack()
        ctx_pool = ab.enter_context(tc.tile_pool(name="ctxsb", bufs=1))
        # ctx lives on partitions 64:128 (pc rows 64:128); rows 0:64 unused
        ctx_sb = [ctx_pool.tile([128, S], BF16, tag=f"c{i}", name=f"ctxsb{i}") for i in range(HL)]
        consts = ab.enter_context(tc.tile_pool(name="consts", bufs=1))
        triu = consts.tile([128, 128], BF16)
        bq = [consts.tile([128, 1], F32, tag=f"bq{j}", name=f"bq{j}") for j in range(3)]

        # persistent qkv activations (phases A+B)
        qkv_pool = ab.enter_context(tc.tile_pool(name="qkv", bufs=1))
        q_sb = [qkv_pool.tile([128, S], BF16, tag=f"q{i}", name=f"qsb{i}") for i in range(HL // 2)]
        kv_sb = qkv_pool.tile([128, S], BF16, tag="kv")      # k rows 0:64
        kdup = qkv_pool.tile([128, S], BF16, tag="kdup")     # roped k duplicated
        v_pool = ab.enter_context(tc.tile_pool(name="vsb", bufs=1))
        # v tiles: cols 0:64 = ones (denominator trick), cols 64:128 = v^T
        v_sb = [v_pool.tile([128, 2 * D], BF16, tag=f"v{t}", name=f"vsb{t}") for t in range(NT)]

        # out-projection weight tiles persist from mid-attention to phase C
        wo_pool = ctx.enter_context(tc.tile_pool(name="wo", bufs=1, side="right"))

        def _wo_tile(f):
            w = wo_pool.tile([128, E], BF16, tag=f"wo{f}", name=f"wo{f}")
            wo_sb[f] = w
            nc.sync.dma_start(w[:], woT[f * 128:(f + 1) * 128, :])

        # ---------------- Phase A: QKV + RoPE + V transpose -------------
        with tc.tile_pool(name="xw", bufs=1) as xw_pool, \
             tc.tile_pool(name="ropet", bufs=1) as rope_pool, \
             tc.tile_pool(name="psqkv", bufs=2, space="PSUM") as ps_qkv, \
             tc.tile_pool(name="psvt", bufs=2, space="PSUM") as ps_vt:

            ident = xw_pool.tile([128, 64], F32)
            wq_sb = [xw_pool.tile([128, OPL], BF16, tag=f"wq{e}", name=f"wqsb{e}") for e in range(NE)]
            # x^T streamed per half as [128, 1024] column slices (each byte
            # read once, 2KB per partition descriptor); double-buffered so
            # half 1 loads during half 0's matmuls.
            xs = {}

            def _x_tile(h, e):
                xs[(h, e)] = xw_pool.tile([128, 1024], BF16, tag=f"x{e}",
                                          bufs=2, name=f"xs{h}_{e}")
                nc.sync.dma_start(
                    xs[(h, e)][:],
                    xT[e * 128:(e + 1) * 128, h * 1024:(h + 1) * 1024])

            # issue order: exactly what the first matmuls need first
            for e in range(2):
                nc.sync.dma_start(wq_sb[e][:], wqkvT[e * 128:(e + 1) * 128, :])
                _x_tile(0, e)
            for j in range(3):
                nc.sync.dma_start(bq[j][:], bqkv[j * 128:(j + 1) * 128, :])
            nc.sync.dma_start(ident[:], id_d[:])
            # cc/ss streamed per half alongside x; vstage per half (fp32 on
            # partitions 64:128 so the PE transpose runs the exact
            # fp32/base-64 pattern known to work on hardware)
            ccss = {}

            def _ccss(h):
                for nm, src in (("cc", cc_d), ("ss", ss_d)):
                    tl = xw_pool.tile([128, 1024], BF16, tag=nm, bufs=2)
                    ccss[(h, nm)] = tl
                    nc.sync.dma_start(tl[:], src[:, h * 1024:(h + 1) * 1024])

            for e in range(2, NE):
                nc.sync.dma_start(wq_sb[e][:], wqkvT[e * 128:(e + 1) * 128, :])
                _x_tile(0, e)
            _ccss(0)
            nc.sync.dma_start(triu[:], triu_d[:])

            # warm the ACT exp table so the first attention exp call does
            # not pay the ACT_TABLE_LOAD at the QKV->attention boundary
            warm = xw_pool.tile([128, 1], F32, tag="warm")
            nc.scalar.activation(warm[:], bq[0][:], AF.Exp, scale=0.0)
            # ones columns of the v tiles (written once, reused throughout)
            for t in range(NT):
                nc.vector.memset(v_sb[t][:, 0:D], 1.0)

            for h in range(2):          # halves of the sequence (1024 cols)
                hs = slice(h * 1024, (h + 1) * 1024)
                if h == 1:
                    for e in range(NE):
                        _x_tile(1, e)
                    _ccss(1)
                vstage = xw_pool.tile([128, 1024], F32, tag="vstage", bufs=2)
                cc = ccss[(h, "cc")]
                ss = ccss[(h, "ss")]
                dsts = [q_sb[0], q_sb[1], kv_sb]
                for j in range(3):
                    ps = ps_qkv.tile([128, 1024], F32, tag="qkvps")
                    for e in range(NE):
                        for ch in range(2):
                            nc.tensor.matmul(
                                ps[:, ch * 512:(ch + 1) * 512],
                                wq_sb[e][:, j * 128:(j + 1) * 128],
                                xs[(h, e)][:, ch * 512:(ch + 1) * 512],
                                start=(e == 0), stop=(e == NE - 1))
                    if j < 2:
                        nc.scalar.activation(
                            dsts[j][:, hs], ps[:],
                            AF.Identity, bias=bq[j][:], scale=1.0)
                    else:
                        nc.scalar.activation(
                            kv_sb[0:64, hs], ps[0:64, :],
                            AF.Identity, bias=bq[2][0:64], scale=1.0)
                        nc.scalar.activation(
                            vstage[64:128, :], ps[64:128, :],
                            AF.Identity, bias=bq[2][64:128], scale=1.0)
                # ---- RoPE per 512-column chunk of this half ----
                for c2 in range(2):
                    sg = 2 * h + c2
                    sgs = slice(sg * 512, (sg + 1) * 512)
                    rs = slice(c2 * 512, (c2 + 1) * 512)      # half-relative
                    for qi in range(HL // 2):
                        qt = q_sb[qi]
                        qs = rope_pool.tile([128, 512], BF16, tag="qs")
                        for b in range(4):
                            nc.gpsimd.dma_start(
                                qs[b * 32:(b + 1) * 32, :],
                                qt[(b ^ 1) * 32:((b ^ 1) + 1) * 32, sgs])
                        t1 = rope_pool.tile([128, 512], BF16, tag="t1")
                        t2 = rope_pool.tile([128, 512], BF16, tag="t2")
                        nc.vector.tensor_mul(t1[:], qt[:, sgs], cc[:, rs])
                        nc.vector.tensor_mul(t2[:], qs[:], ss[:, rs])
                        nc.vector.tensor_add(qt[:, sgs], t1[:], t2[:])
                    # k: rows 0:64 of kv_sb -> roped into kdup, then dup'd
                    ks = rope_pool.tile([64, 512], BF16, tag="ks")
                    for b in range(2):
                        nc.gpsimd.dma_start(
                            ks[b * 32:(b + 1) * 32, :],
                            kv_sb[(b ^ 1) * 32:((b ^ 1) + 1) * 32, sgs])
                    t1 = rope_pool.tile([64, 512], BF16, tag="kt1")
                    t2 = rope_pool.tile([64, 512], BF16, tag="kt2")
                    nc.vector.tensor_mul(t1[:], kv_sb[0:64, sgs], cc[0:64, rs])
                    nc.vector.tensor_mul(t2[:], ks[:], ss[0:64, rs])
                    nc.vector.tensor_add(kdup[0:64, sgs], t1[:], t2[:])
                    nc.gpsimd.dma_start(kdup[64:128, sgs], kdup[0:64, sgs])
                    # ---- V transpose: vstage rows 64:128 -> v_sb tiles ----
                    for c in range(4):
                        t = sg * 4 + c
                        pvt = ps_vt.tile([128, 64], F32, tag="vt")
                        nc.tensor.transpose(
                            pvt[:],
                            vstage[64:128, c2 * 512 + c * 128: c2 * 512 + (c + 1) * 128],
                            ident[64:128, :])
                        nc.vector.tensor_copy(v_sb[t][:, D:2 * D], pvt[:])

        # ---------------- Phase B: attention --------------------------
        with tc.tile_pool(name="exps", bufs=2) as exps_pool, \
             tc.tile_pool(name="rcp", bufs=2) as rcp_pool, \
             tc.tile_pool(name="pss", bufs=2, space="PSUM") as ps_s, \
             tc.tile_pool(name="psctx", bufs=2, space="PSUM") as ps_ctx:
            dram = ctx.enter_context(tc.tile_pool(name="dram", bufs=1, space="DRAM"))
            a2a_in = [dram.tile([n_cores * 128, SEG], BF16, tag=f"ai{p}", name=f"a2ain{p}")
                      for p in range(2)]
            a2a_out = [dram.tile([n_cores * 128, SEG], BF16, tag=f"ao{p}", name=f"a2aout{p}")
                       for p in range(2)]
            outb = None
            for hp in range(HL // 2):
                qt = q_sb[hp]
                for g in range(NG):
                    # stream 2 of the 16 out-proj weight tiles per segment
                    seg_i = hp * NG + g
                    _wo_tile(2 * seg_i)
                    _wo_tile(2 * seg_i + 1)
                    if seg_i == 4:
                        outb = wo_pool.tile([128, E], F32, tag="outb")
                        nc.scalar.dma_start(outb[:], outb_d[:])
                    gs = slice(g * 512, (g + 1) * 512)
                    pc = ps_ctx.tile([128, 1024], F32, tag="ctx")
                    ntb = 4 * g + 4
                    for t in range(ntb):
                        ts_ = slice(t * 128, (t + 1) * 128)
                        j = t - 4 * g
                        c0 = max(0, j) * 128
                        pss = ps_s.tile([128, 1024], F32, tag="s")
                        nc.tensor.matmul(
                            pss[:, c0:512], kdup[0:64, ts_],
                            qt[0:64, g * 512 + c0:(g + 1) * 512],
                            start=True, stop=True)
                        nc.tensor.matmul(
                            pss[:, 512 + c0:1024], kdup[64:128, ts_],
                            qt[64:128, g * 512 + c0:(g + 1) * 512],
                            start=True, stop=True)
                        ex = exps_pool.tile([128, 1024], BF16, tag="e")
                        if j <= 0:
                            nc.scalar.activation(ex[:], pss[:], AF.Exp,
                                                 scale=SCALE)
                        else:
                            # valid cols only: [c0:512] of each half
                            ex_r = ex[:].rearrange("p (k c) -> p k c", k=2)
                            ps_r = pss[:].rearrange("p (k c) -> p k c", k=2)
                            nc.scalar.activation(ex_r[:, :, c0:], ps_r[:, :, c0:],
                                                 AF.Exp, scale=SCALE)
                            # stale cols must be zero for the full-width AV
                            nc.vector.memset(ex_r[:, :, :c0], 0.0)
                        if j >= 0:
                            for h2 in range(2):
                                sl = slice(h2 * 512 + c0, h2 * 512 + c0 + 128)
                                nc.vector.tensor_mul(ex[:, sl], ex[:, sl], triu[:])
                        nc.tensor.matmul(pc[:, 0:512], v_sb[t][:], ex[:, 0:512],
                                         start=(t == 0), stop=(t == ntb - 1))
                        nc.tensor.matmul(pc[:, 512:1024], v_sb[t][:],
                                         ex[:, 512:1024],
                                         start=(t == 0), stop=(t == ntb - 1))
                    # softmax denominators sit on pc rows 0:64 (64 copies);
                    # reciprocal at base 0 (the proven custom-DVE config),
                    # then one DMA realigns it to the ctx rows 64:128
                    rb = rcp_pool.tile([64, 1024], F32, tag="rb")
                    rscr = rcp_pool.tile([64, 1024], F32, tag="rscr")
                    nc.vector.reciprocal_approx_accurate(
                        rb[:], pc[0:64, :], rscr[:])
                    rbh = rcp_pool.tile([128, 1024], F32, tag="rbh")
                    nc.gpsimd.dma_start(rbh[64:128, :], rb[:])
                    nc.vector.tensor_mul(ctx_sb[2 * hp][64:128, gs],
                                         pc[64:128, 0:512], rbh[64:128, 0:512])
                    nc.vector.tensor_mul(ctx_sb[2 * hp + 1][64:128, gs],
                                         pc[64:128, 512:1024], rbh[64:128, 512:1024])
                # this head-pair's ctx rows are final: ship its AllToAll now
                # so the collective overlaps the next head-pair's attention
                for jj in range(n_cores):
                    for h2 in range(2):
                        nc.gpsimd.dma_start(
                            a2a_in[hp][jj * 128 + h2 * 64: jj * 128 + (h2 + 1) * 64, :],
                            ctx_sb[2 * hp + h2][64:128, jj * SEG:(jj + 1) * SEG])
                with tc.high_priority():
                    nc.gpsimd.collective_compute(
                        "AllToAll", ALU.bypass,
                        replica_groups=[list(range(n_cores))],
                        ins=[a2a_in[hp][:]], outs=[a2a_out[hp][:]])

        ab.close()  # release qkv/v/consts SBUF before the out-projection

        # ---------------- Phase C: out projection ---------------------
        with tc.tile_pool(name="cfx", bufs=1) as cfx_pool, \
             tc.tile_pool(name="osb", bufs=1) as out_pool, \
             tc.tile_pool(name="pso", bufs=2, space="PSUM") as ps_o:
            ctxF = [cfx_pool.tile([128, SEG], BF16, tag=f"cf{f}", name=f"cfsb{f}") for f in range(NE)]

            def _ctxF(par):
                for f in range(par, NE, 2):
                    c = f // 2
                    nc.scalar.dma_start(
                        ctxF[f][:], a2a_out[par][c * 128:(c + 1) * 128, :])

            _ctxF(0)
            _ctxF(1)
            out_sb = [out_pool.tile([128, E], F32, tag=f"ot{s}", name=f"osb{s}")
                      for s in range(SEG // 128)]
            # pass 1: head-pair-0 partial products (only need A2A #0), added
            # into out_sb with the bias; pass 2 accumulates head-pair-1 on
            # top once A2A #1 has landed. This keeps the PE busy during the
            # second collective.
            groups = [(ehalf, egl, st) for ehalf in range(2)
                      for egl in range(2) for st in range(SEG // 128)]
            for p in range(2):
                for (ehalf, egl, st) in groups:
                    eg = ehalf * 2 + egl
                    po = ps_o.tile([128, 512], F32, tag=f"o{p}")
                    fs = [f for f in range(NE) if f % 2 == p]
                    for fi, f in enumerate(fs):
                        nc.tensor.matmul(
                            po[:],
                            ctxF[f][:, st * 128:(st + 1) * 128],
                            wo_sb[f][:, eg * 512:(eg + 1) * 512],
                            start=(fi == 0), stop=(fi == len(fs) - 1))
                    osl = out_sb[st][:, eg * 512:(eg + 1) * 512]
                    if p == 0:
                        nc.vector.scalar_tensor_tensor(
                            osl, po[:], 1.0,
                            outb[:, eg * 512:(eg + 1) * 512], ALU.mult, ALU.add)
                    else:
                        nc.vector.scalar_tensor_tensor(
                            osl, po[:], 1.0, osl, ALU.mult, ALU.add)
            for st in range(SEG // 128):
                nc.sync.dma_start(outS[st * 128:(st + 1) * 128, :], out_sb[st][:])

    nc.compile()
    return nc


def shard_inputs(hidden_states, Wqkv_w, Wqkv_b, out_w, out_b, S=2048,
                 n_cores=NCORES):
    """Host-side sharding: returns per-core input maps."""
    bf16 = ml_dtypes.bfloat16
    x = np.asarray(hidden_states, np.float32).reshape(S, E)
    xT = np.ascontiguousarray(x.T).astype(bf16)
    Wqkv_w = np.asarray(Wqkv_w, np.float32)
    Wqkv_b = np.asarray(Wqkv_b, np.float32)
    woT = np.ascontiguousarray(np.asarray(out_w, np.float32).T).astype(bf16)
    outb = np.ascontiguousarray(np.broadcast_to(
        np.asarray(out_b, np.float32).reshape(1, E), (128, E)))

    inv = (1.0 / (BASE ** (np.arange(0, ROT, 2, dtype=np.float64) / ROT)))
    t = np.arange(S, dtype=np.float64)
    freqs = np.outer(t, inv)                      # [S, 32]
    cT = np.cos(freqs).T.astype(np.float32)       # [32, S]
    sT = np.sin(freqs).T.astype(np.float32)
    cc = np.tile(cT, (4, 1)).astype(bf16)         # [128, S]
    ss = np.concatenate([-sT, sT, -sT, sT], axis=0).astype(bf16)
    triu = (np.arange(128)[:, None] <= np.arange(128)[None, :]).astype(bf16)
    ident = np.vstack([np.eye(64, dtype=np.float32)] * 2)

    in_maps = []
    for i in range(n_cores):
        hq = H // n_cores
        wq = Wqkv_w[i * hq * D:(i + 1) * hq * D]          # [256, E]
        wk = Wqkv_w[H * D + i * D: H * D + (i + 1) * D]   # [64, E]
        wv = Wqkv_w[(H + HKV) * D + i * D: (H + HKV) * D + (i + 1) * D]
        w_local = np.concatenate([wq, wk, wv], axis=0)    # [384, E]
        b_local = np.concatenate([
            Wqkv_b[i * hq * D:(i + 1) * hq * D],
            Wqkv_b[H * D + i * D: H * D + (i + 1) * D],
            Wqkv_b[(H + HKV) * D + i * D: (H + HKV) * D + (i + 1) * D]])
        in_maps.append({
            "xT": xT,
            "wqkvT": np.ascontiguousarray(w_local.T).astype(bf16),
            "bqkv": np.ascontiguousarray(b_local.reshape(OPL, 1)),
            "cc": cc, "ss": ss, "triu": triu, "ident": ident,
            "woT": woT, "outb": outb,
        })
    return in_maps


def assemble(results, S=2048, n_cores=NCORES):
    out = np.concatenate([r["outS"] for r in results], axis=0)
    return out.reshape(B, S, E).astype(np.float32)


_NC_CACHE = {}


def _get_nc(S=2048):
    if S not in _NC_CACHE:
        _NC_CACHE[S] = build_nc(S=S)
    return _NC_CACHE[S]


def kernel(hidden_states, Wqkv_w, Wqkv_b, out_w, out_b, _trace=False):
    in_maps = shard_inputs(hidden_states, Wqkv_w, Wqkv_b, out_w, out_b)
    nc = _get_nc()
    res = run_bass_kernel_spmd(nc, in_maps, core_ids=list(range(NCORES)),
                               trace=_trace)
    out = assemble(res.results)
    if _trace:
        kernel.last_results = res
    return out


# revision 27
# speedup vs baseline: 1.3818x; 1.0507x over previous
"""Trainium2 Bass kernel for nn_MHA_42391327211690.

MHA: B=1, S=2048, E=2048, H=32 q-heads, HKV=8 kv-heads, D=64, RoPE(rot=64,
GPT-NeoX style) on q/k, causal GQA attention, out-projection with bias.

Distribution (8 NeuronCores, tensor-parallel by heads):
  - core i computes q-heads 4i..4i+3 and kv-head i (Wqkv column-sharded),
  - attention entirely local (GQA groups align with the shard),
  - AllToAll redistributes ctx^T from head-sharded to sequence-sharded,
  - out-projection computed per-core for its 256-row sequence slice
    (weights replicated), host concatenates the slices.

v3 schedule: everything bf16 (host-converted); QKV halves interleaved with
the first head-pair's attention groups so the x DMA and QKV matmuls hide
under attention and the PE stays p-state-ramped; causal mask applied ON THE
PE (a -1e9 upper-triangle accumulated into the diagonal score blocks via
mask^T @ I) so the scores->exp->AV chain never waits on the vector engine;
softmax denominator comes from 64 ones-columns in the V tiles (AV matmul
rows 0:64 = den, 64:128 = ctx); out-proj weights stream during attention;
out-proj pass 1 (head-pair 0) runs under the second AllToAll.
"""

from contextlib import ExitStack

import numpy as np
import ml_dtypes

import concourse.bass as bass
import concourse.bacc as bacc
import concourse.tile as tile
from concourse import mybir
from concourse.bass_utils import run_bass_kernel_spmd

F32 = mybir.dt.float32
BF16 = mybir.dt.bfloat16
AF = mybir.ActivationFunctionType
ALU = mybir.AluOpType

B, E = 1, 2048
H, HKV, D = 32, 8, 64
ROT, BASE = 64, 10000.0
NCORES = 8
HL = H // NCORES            # 4 local q heads
OPL = (HL + 2) * D          # 384 local qkv output rows (q | k | v)
SCALE = float(D) ** -0.5


def build_nc(S=2048, n_cores=NCORES):
    """Build the SPMD Bass program (identical on every core)."""
    SEG = S // n_cores      # per-core output sequence slice
    NT = S // 128           # t-blocks (key blocks)
    NG = S // 512           # sq groups of 512
    NE = E // 128           # contraction tiles for qkv / out proj

    nc = bacc.Bacc("TRN2", target_bir_lowering=False, debug=False,
                   num_devices=n_cores)

    xT = nc.dram_tensor("xT", [E, S], BF16, kind="ExternalInput")
    wqkvT = nc.dram_tensor("wqkvT", [E, OPL], BF16, kind="ExternalInput")
    bqkv = nc.dram_tensor("bqkv", [OPL, 1], F32, kind="ExternalInput")
    cc_d = nc.dram_tensor("cc", [128, S], BF16, kind="ExternalInput")
    ss_d = nc.dram_tensor("ss", [128, S], BF16, kind="ExternalInput")
    trm_d = nc.dram_tensor("trm", [128, 128], BF16, kind="ExternalInput")
    id128_d = nc.dram_tensor("id128", [128, 128], BF16, kind="ExternalInput")
    id_d = nc.dram_tensor("ident", [128, 64], F32, kind="ExternalInput")
    woT = nc.dram_tensor("woT", [E, E], BF16, kind="ExternalInput")
    outb_d = nc.dram_tensor("outb", [128, E], F32, kind="ExternalInput")
    outS = nc.dram_tensor("outS", [SEG, E], F32, kind="ExternalOutput")

    with tile.TileContext(nc) as tc, ExitStack() as ctx:
        ab = ExitStack()
        ctx_pool = ab.enter_context(tc.tile_pool(name="ctxsb", bufs=1))
        # ctx lives on partitions 64:128 (pc rows 64:128); rows 0:64 unused
        ctx_sb = [ctx_pool.tile([128, S], BF16, tag=f"c{i}", name=f"ctxsb{i}") for i in range(HL)]
        consts = ab.enter_context(tc.tile_pool(name="consts", bufs=1))
        trm = consts.tile([128, 128], BF16)       # -1e9 upper-strict^T mask
        id128 = consts.tile([128, 128], BF16)
        bq = [consts.tile([128, 1], F32, tag=f"bq{j}", name=f"bq{j}") for j in range(3)]

        # persistent qkv activations
        qkv_pool = ab.enter_context(tc.tile_pool(name="qkv", bufs=1))
        q_sb = [qkv_pool.tile([128, S], BF16, tag=f"q{i}", name=f"qsb{i}") for i in range(HL // 2)]
        kv_sb = qkv_pool.tile([128, S], BF16, tag="kv")      # k rows 0:64
        kdup = qkv_pool.tile([128, S], BF16, tag="kdup")     # roped k duplicated
        v_pool = ab.enter_context(tc.tile_pool(name="vsb", bufs=1))
        # v tiles: cols 0:64 = ones (denominator trick), cols 64:128 = v^T
        v_sb = [v_pool.tile([128, 2 * D], BF16, tag=f"v{t}", name=f"vsb{t}") for t in range(NT)]

        # right side: out-proj weights + bias + ctxF persist into phase C
        wo_pool = ctx.enter_context(tc.tile_pool(name="wo", bufs=1, side="right"))
        wo_sb = {}
        ctxF = [None] * NE
        outb = [None]

        def _wo_tile(f):
            w = wo_pool.tile([128, E], BF16, tag=f"wo{f}", name=f"wo{f}")
            wo_sb[f] = w
            nc.sync.dma_start(w[:], woT[f * 128:(f + 1) * 128, :])

        def _ctxF(par, a2a_out):
            for f in range(par, NE, 2):
                c = f // 2
                t = wo_pool.tile([128, SEG], BF16, tag=f"cf{f}", name=f"cfsb{f}")
                ctxF[f] = t
                nc.scalar.dma_start(t[:], a2a_out[par][c * 128:(c + 1) * 128, :])

        # ---------------- phases A+B share these pools ------------------
        with tc.tile_pool(name="exps", bufs=2) as exps_pool, \
             tc.tile_pool(name="rcp", bufs=2) as rcp_pool, \
             tc.tile_pool(name="psmm", bufs=2, space="PSUM") as ps_mm, \
             tc.tile_pool(name="psctx", bufs=2, space="PSUM") as ps_ctx:
            dram = ctx.enter_context(tc.tile_pool(name="dram", bufs=1, space="DRAM"))
            a2a_in = [dram.tile([n_cores * 128, SEG], BF16, tag=f"ai{p}", name=f"a2ain{p}")
                      for p in range(2)]
            a2a_out = [dram.tile([n_cores * 128, SEG], BF16, tag=f"ao{p}", name=f"a2aout{p}")
                       for p in range(2)]

            def attn_group(hp, g):
                qt = q_sb[hp]
                gs = slice(g * 512, (g + 1) * 512)
                pc = ps_ctx.tile([128, 1024], F32, tag="ctx")
                ntb = 4 * g + 4
                for t in range(ntb):
                    ts_ = slice(t * 128, (t + 1) * 128)
                    j = t - 4 * g
                    c0 = max(0, j) * 128
                    pss = ps_mm.tile([128, 1024], F32, tag="mm")
                    nc.tensor.matmul(
                        pss[:, c0:512], kdup[0:64, ts_],
                        qt[0:64, g * 512 + c0:(g + 1) * 512],
                        start=True, stop=(j < 0))
                    nc.tensor.matmul(
                        pss[:, 512 + c0:1024], kdup[64:128, ts_],
                        qt[64:128, g * 512 + c0:(g + 1) * 512],
                        start=True, stop=(j < 0))
                    if j >= 0:
                        # causal mask on the PE: accumulate -1e9 strict upper
                        # triangle (trm^T) onto the diagonal 128-col block
                        nc.tensor.matmul(pss[:, c0:c0 + 128], trm[:], id128[:],
                                         start=False, stop=True)
                        nc.tensor.matmul(pss[:, 512 + c0:512 + c0 + 128],
                                         trm[:], id128[:],
                                         start=False, stop=True)
                    ex = exps_pool.tile([128, 1024], BF16, tag="e")
                    if j <= 0:
                        nc.scalar.activation(ex[:], pss[:], AF.Exp, scale=SCALE)
                    else:
                        ex_r = ex[:].rearrange("p (k c) -> p k c", k=2)
                        ps_r = pss[:].rearrange("p (k c) -> p k c", k=2)
                        nc.scalar.activation(ex_r[:, :, c0:], ps_r[:, :, c0:],
                                             AF.Exp, scale=SCALE)
                        if j == 3:
                            # last block is full-width (carries the stop
                            # flag): its unread cols must be zeros
                            nc.vector.memset(ex_r[:, :, :c0], 0.0)
                    # AV: restricted width for inner diagonal blocks (pure
                    # accumulate), full width on first and last blocks
                    if j in (1, 2):
                        nc.tensor.matmul(pc[:, c0:512], v_sb[t][:],
                                         ex[:, c0:512],
                                         start=False, stop=False)
                        nc.tensor.matmul(pc[:, 512 + c0:1024], v_sb[t][:],
                                         ex[:, 512 + c0:1024],
                                         start=False, stop=False)
                    else:
                        nc.tensor.matmul(pc[:, 0:512], v_sb[t][:],
                                         ex[:, 0:512],
                                         start=(t == 0), stop=(t == ntb - 1))
                        nc.tensor.matmul(pc[:, 512:1024], v_sb[t][:],
                                         ex[:, 512:1024],
                                         start=(t == 0), stop=(t == ntb - 1))
                # denominators on pc rows 0:64 (64 copies); reciprocal at
                # base 0, one DMA realigns to the ctx rows 64:128
                rb = rcp_pool.tile([64, 1024], F32, tag="rb")
                rscr = rcp_pool.tile([64, 1024], F32, tag="rscr")
                nc.vector.reciprocal_approx_accurate(rb[:], pc[0:64, :], rscr[:])
                rbh = rcp_pool.tile([128, 1024], F32, tag="rbh")
                nc.gpsimd.dma_start(rbh[64:128, :], rb[:])
                nc.vector.tensor_mul(ctx_sb[2 * hp][64:128, gs],
                                     pc[64:128, 0:512], rbh[64:128, 0:512])
                nc.vector.tensor_mul(ctx_sb[2 * hp + 1][64:128, gs],
                                     pc[64:128, 512:1024], rbh[64:128, 512:1024])

            def ship_a2a(hp):
                for jj in range(n_cores):
                    for h2 in range(2):
                        nc.gpsimd.dma_start(
                            a2a_in[hp][jj * 128 + h2 * 64: jj * 128 + (h2 + 1) * 64, :],
                            ctx_sb[2 * hp + h2][64:128, jj * SEG:(jj + 1) * SEG])
                with tc.high_priority():
                    nc.gpsimd.collective_compute(
                        "AllToAll", ALU.bypass,
                        replica_groups=[list(range(n_cores))],
                        ins=[a2a_in[hp][:]], outs=[a2a_out[hp][:]])

            # ------------- Phase A: QKV + RoPE + V transpose -------------
            with tc.tile_pool(name="xw", bufs=1) as xw_pool, \
                 tc.tile_pool(name="ropet", bufs=1) as rope_pool:
                ident = xw_pool.tile([128, 64], F32)
                wq_sb = [xw_pool.tile([128, OPL], BF16, tag=f"wq{e}", name=f"wqsb{e}") for e in range(NE)]
                xs = {}
                ccss = {}

                def _x_tile(h, e):
                    xs[(h, e)] = xw_pool.tile([128, 1024], BF16, tag=f"x{e}",
                                              bufs=1, name=f"xs{h}_{e}")
                    nc.sync.dma_start(
                        xs[(h, e)][:],
                        xT[e * 128:(e + 1) * 128, h * 1024:(h + 1) * 1024])

                def _ccss(h):
                    for nm, src in (("cc", cc_d), ("ss", ss_d)):
                        tl = xw_pool.tile([128, 1024], BF16, tag=nm, bufs=1)
                        ccss[(h, nm)] = tl
                        nc.sync.dma_start(tl[:], src[:, h * 1024:(h + 1) * 1024])

                # issue order: exactly what the first matmuls need first
                for e in range(2):
                    nc.sync.dma_start(wq_sb[e][:], wqkvT[e * 128:(e + 1) * 128, :])
                    _x_tile(0, e)
                for j in range(3):
                    nc.sync.dma_start(bq[j][:], bqkv[j * 128:(j + 1) * 128, :])
                nc.sync.dma_start(ident[:], id_d[:])
                for e in range(2, NE):
                    nc.sync.dma_start(wq_sb[e][:], wqkvT[e * 128:(e + 1) * 128, :])
                    _x_tile(0, e)
                _ccss(0)
                nc.sync.dma_start(trm[:], trm_d[:])
                nc.sync.dma_start(id128[:], id128_d[:])

                # warm the ACT exp table; ones columns of the v tiles
                warm = xw_pool.tile([128, 1], F32, tag="warm")
                nc.scalar.activation(warm[:], bq[0][:], AF.Exp, scale=0.0)
                for t in range(NT):
                    nc.vector.memset(v_sb[t][:, 0:D], 1.0)

                def qkv_half(h):
                    hs = slice(h * 1024, (h + 1) * 1024)
                    vstage = xw_pool.tile([128, 1024], F32, tag="vstage", bufs=1)
                    cc = ccss[(h, "cc")]
                    ss = ccss[(h, "ss")]
                    dsts = [q_sb[0], q_sb[1], kv_sb]
                    for j in range(3):
                        ps = ps_mm.tile([128, 1024], F32, tag="mm")
                        for e in range(NE):
                            for ch in range(2):
                                nc.tensor.matmul(
                                    ps[:, ch * 512:(ch + 1) * 512],
                                    wq_sb[e][:, j * 128:(j + 1) * 128],
                                    xs[(h, e)][:, ch * 512:(ch + 1) * 512],
                                    start=(e == 0), stop=(e == NE - 1))
                        if j < 2:
                            nc.scalar.activation(
                                dsts[j][:, hs], ps[:],
                                AF.Identity, bias=bq[j][:], scale=1.0)
                        else:
                            nc.scalar.activation(
                                kv_sb[0:64, hs], ps[0:64, :],
                                AF.Identity, bias=bq[2][0:64], scale=1.0)
                            nc.scalar.activation(
                                vstage[64:128, :], ps[64:128, :],
                                AF.Identity, bias=bq[2][64:128], scale=1.0)
                    # ---- RoPE + V transpose per 512-column chunk ----
                    for c2 in range(2):
                        sg = 2 * h + c2
                        sgs = slice(sg * 512, (sg + 1) * 512)
                        rs = slice(c2 * 512, (c2 + 1) * 512)
                        for qi in range(HL // 2):
                            qt = q_sb[qi]
                            qs = rope_pool.tile([128, 512], BF16, tag="qs")
                            for b in range(4):
                                nc.gpsimd.dma_start(
                                    qs[b * 32:(b + 1) * 32, :],
                                    qt[(b ^ 1) * 32:((b ^ 1) + 1) * 32, sgs])
                            t1 = rope_pool.tile([128, 512], BF16, tag="t1")
                            t2 = rope_pool.tile([128, 512], BF16, tag="t2")
                            nc.vector.tensor_mul(t1[:], qt[:, sgs], cc[:, rs])
                            nc.vector.tensor_mul(t2[:], qs[:], ss[:, rs])
                            nc.vector.tensor_add(qt[:, sgs], t1[:], t2[:])
                        ks = rope_pool.tile([64, 512], BF16, tag="ks")
                        for b in range(2):
                            nc.gpsimd.dma_start(
                                ks[b * 32:(b + 1) * 32, :],
                                kv_sb[(b ^ 1) * 32:((b ^ 1) + 1) * 32, sgs])
                        t1 = rope_pool.tile([64, 512], BF16, tag="kt1")
                        t2 = rope_pool.tile([64, 512], BF16, tag="kt2")
                        nc.vector.tensor_mul(t1[:], kv_sb[0:64, sgs], cc[0:64, rs])
                        nc.vector.tensor_mul(t2[:], ks[:], ss[0:64, rs])
                        nc.vector.tensor_add(kdup[0:64, sgs], t1[:], t2[:])
                        nc.gpsimd.dma_start(kdup[64:128, sgs], kdup[0:64, sgs])
                        for c in range(4):
                            t = sg * 4 + c
                            pvt = ps_mm.tile([128, 1024], F32, tag="mm")
                            nc.tensor.transpose(
                                pvt[:, 0:64],
                                vstage[64:128, c2 * 512 + c * 128: c2 * 512 + (c + 1) * 128],
                                ident[64:128, :])
                            nc.vector.tensor_copy(v_sb[t][:, D:2 * D],
                                                  pvt[:, 0:64])

                qkv_half(0)
                # prefetch half 1 while head-pair 0 attends to half 0
                for e in range(NE):
                    _x_tile(1, e)
                _ccss(1)
                attn_group(0, 0)
                attn_group(0, 1)
                qkv_half(1)

            # ------------- Phase B: remaining attention ------------------
            for f in range(4):
                _wo_tile(f)
            attn_group(0, 2)
            for f in range(4, 8):
                _wo_tile(f)
            attn_group(0, 3)
            ship_a2a(0)
            for f in range(8, 12):
                _wo_tile(f)
            attn_group(1, 0)
            for f in range(12, NE):
                _wo_tile(f)
            outb[0] = wo_pool.tile([128, E], F32, tag="outb", name="outb")
            nc.scalar.dma_start(outb[0][:], outb_d[:])
            attn_group(1, 1)
            _ctxF(0, a2a_out)
            attn_group(1, 2)
            attn_group(1, 3)
            ship_a2a(1)
            _ctxF(1, a2a_out)

        ab.close()  # release qkv/v/consts SBUF before the out-projection

        # ---------------- Phase C: out projection ---------------------
        with tc.tile_pool(name="osb", bufs=1) as out_pool, \
             tc.tile_pool(name="pso", bufs=2, space="PSUM") as ps_o:
            out_sb = [out_pool.tile([128, E], F32, tag=f"ot{s}", name=f"osb{s}")
                      for s in range(SEG // 128)]
            # pass 1: head-pair-0 partial products (only need A2A #0), added
            # into out_sb with the bias; pass 2 accumulates head-pair-1 on
            # top once A2A #1 has landed.
            groups = [(ehalf, egl, st) for ehalf in range(2)
                      for egl in range(2) for st in range(SEG // 128)]
            for p in range(2):
                for (ehalf, egl, st) in groups:
                    eg = ehalf * 2 + egl
                    po = ps_o.tile([128, 512], F32, tag=f"o{p}")
                    fs = [f for f in range(NE) if f % 2 == p]
                    for fi, f in enumerate(fs):
                        nc.tensor.matmul(
                            po[:],
                            ctxF[f][:, st * 128:(st + 1) * 128],
                            wo_sb[f][:, eg * 512:(eg + 1) * 512],
                            start=(fi == 0), stop=(fi == len(fs) - 1))
                    osl = out_sb[st][:, eg * 512:(eg + 1) * 512]
                    if p == 0:
                        nc.vector.scalar_tensor_tensor(
                            osl, po[:], 1.0,
                            outb[0][:, eg * 512:(eg + 1) * 512], ALU.mult, ALU.add)
                    else:
                        nc.vector.scalar_tensor_tensor(
                            osl, po[:], 1.0, osl, ALU.mult, ALU.add)
            for st in range(SEG // 128):
                nc.sync.dma_start(outS[st * 128:(st + 1) * 128, :], out_sb[st][:])

    nc.compile()
    return nc


def shard_inputs(hidden_states, Wqkv_w, Wqkv_b, out_w, out_b, S=2048,
                 n_cores=NCORES):
    """Host-side sharding: returns per-core input maps."""
    bf16 = ml_dtypes.bfloat16
    x = np.asarray(hidden_states, np.float32).reshape(S, E)
    xT = np.ascontiguousarray(x.T).astype(bf16)
    Wqkv_w = np.asarray(Wqkv_w, np.float32)
    Wqkv_b = np.asarray(Wqkv_b, np.float32)
    woT = np.ascontiguousarray(np.asarray(out_w, np.float32).T).astype(bf16)
    outb = np.ascontiguousarray(np.broadcast_to(
        np.asarray(out_b, np.float32).reshape(1, E), (128, E)))

    inv = (1.0 / (BASE ** (np.arange(0, ROT, 2, dtype=np.float64) / ROT)))
    t = np.arange(S, dtype=np.float64)
    freqs = np.outer(t, inv)                      # [S, 32]
    cT = np.cos(freqs).T.astype(np.float32)       # [32, S]
    sT = np.sin(freqs).T.astype(np.float32)
    cc = np.tile(cT, (4, 1)).astype(bf16)         # [128, S]
    ss = np.concatenate([-sT, sT, -sT, sT], axis=0).astype(bf16)
    # trm[k, m] = -1e9 where key row m > query col k would be masked:
    # accumulated as trm^T @ I, giving scores[t, c] += trm[c, t]
    mask = (np.arange(128)[:, None] > np.arange(128)[None, :])  # t > c strict
    trm = np.ascontiguousarray((mask.T * (-1e9)).astype(np.float32)).astype(bf16)
    id128 = np.eye(128, dtype=np.float32).astype(bf16)
    ident = np.vstack([np.eye(64, dtype=np.float32)] * 2)

    in_maps = []
    for i in range(n_cores):
        hq = H // n_cores
        wq = Wqkv_w[i * hq * D:(i + 1) * hq * D]          # [256, E]
        wk = Wqkv_w[H * D + i * D: H * D + (i + 1) * D]   # [64, E]
        wv = Wqkv_w[(H + HKV) * D + i * D: (H + HKV) * D + (i + 1) * D]
        w_local = np.concatenate([wq, wk, wv], axis=0)    # [384, E]
        b_local = np.concatenate([
            Wqkv_b[i * hq * D:(i + 1) * hq * D],
            Wqkv_b[H * D + i * D: H * D + (i + 1) * D],
            Wqkv_b[(H + HKV) * D + i * D: (H + HKV) * D + (i + 1) * D]])
        in_maps.append({
            "xT": xT,
            "wqkvT": np.ascontiguousarray(w_local.T).astype(bf16),
            "bqkv": np.ascontiguousarray(b_local.reshape(OPL, 1)),
            "cc": cc, "ss": ss, "trm": trm, "id128": id128, "ident": ident,
            "woT": woT, "outb": outb,
        })
    return in_maps


def assemble(results, S=2048, n_cores=NCORES):
    out = np.concatenate([r["outS"] for r in results], axis=0)
    return out.reshape(B, S, E).astype(np.float32)


_NC_CACHE = {}


def _get_nc(S=2048):
    if S not in _NC_CACHE:
        _NC_CACHE[S] = build_nc(S=S)
    return _NC_CACHE[S]


def kernel(hidden_states, Wqkv_w, Wqkv_b, out_w, out_b, _trace=False):
    in_maps = shard_inputs(hidden_states, Wqkv_w, Wqkv_b, out_w, out_b)
    nc = _get_nc()
    res = run_bass_kernel_spmd(nc, in_maps, core_ids=list(range(NCORES)),
                               trace=_trace)
    out = assemble(res.results)
    if _trace:
        kernel.last_results = res
    return out
